# revision 10
# baseline (speedup 1.0000x reference)
"""Multi-head attention kernel for 8 Trainium2 NeuronCores (v2).

Problem: B=4, L=2048, DIM=1024, H=16 heads, d_k=d_v=64.
  qh = q @ Wq_h ; kh = k @ Wk_h ; vh = v @ Wv_h          (per head)
  out_h = softmax(qh kh^T / sqrt(DIM)) vh
  y = concat_h(out_h) @ proj_w.T + proj_b

Sharding: data-parallel over (batch, query-half): core c handles batch
c//2, query rows [1024*(c%2), ...+1024). The full K/V of the batch is
REPLICATED to both cores of the pair at input-staging time (untimed),
so there are no collectives; each core projects all 2048 kv tokens
itself (+55us tensor vs split-proj, but zero CC stalls).

Per-core dataflow (matmul inputs bf16, fp32 PSUM accum):
  K-proj: kht[hp] [128=dk-pair, 2048s] = sum_d wk[d,hp-cols].T @ kT[d]
  Q-proj: qht[hp] [128=dk-pair, 1024q] likewise (NO zero padding)
  V-proj: vhp[st] [128s, 16h, 64+1] = vT-chunk.T @ wv (+ ones col),
      split into head-halves (lo before the hp0..3 pipeline, hi before
      hp4..7) so the ACT engine starts on exp early.
  scores (hp, kt): TWO CONCURRENT row-tiled matmuls (K=64 each):
      head A = PE rows 0:64 -> scA[128s,1024q], head B = rows 64:128
      -> scB (tile_position auto-derived from base partitions).
  exp via ACT scale=1/32; PV trails per kt: op{A,B}_{q0,q1}[0:65,512]
      += vhp[kt][:,h,:].T @ exp; row 64 = softmax denominator.
  norm: recip -> DRAM bounce -> partition-bcast -> mul. Head A lands
      in oall[hp][0:64]; head B normalized into a tmp tile then
      SBUF->SBUF DMA'd to oall[hp][64:128] (engines can't
      partition-shift; DMA can).
  C: yT[dt] = sum_hp pwsb[hp][:,dt-chunk].T @ oall[hp] + bias.
      oall is PACKED (both heads real) so C uses 8 full-K=128 chunks.

PSUM (8 banks): scA(2) + scB(2) + opA(2) + opB(2); projections and
phase C share the scA/scB buffers via pool tags.
"""

import numpy as np

P = 128
B, L, DIM, H, DK = 4, 2048, 1024, 16, 64
TQ = 1024      # q tokens per core
TS = 2048      # kv tokens per core (full batch)
NDCH = DIM // P          # 8 contraction chunks
NHP = H // 2             # 8 head pairs
NST = TS // P            # 16 key tiles
N_CORES = 8

_NC = None
TRACE = False
LAST_RESULT = None


def _build():
    import concourse.bass as bass
    from concourse import bacc
    import concourse.mybir as mybir
    import concourse.tile as tile

    DT_B = mybir.dt.bfloat16
    DT_F = mybir.dt.float32
    AF = mybir.ActivationFunctionType

    nc = bacc.Bacc(None, target_bir_lowering=False)
    qT = nc.dram_tensor("qT", [DIM, TQ], DT_B, kind="ExternalInput")
    kT = nc.dram_tensor("kT", [DIM, TS], DT_B, kind="ExternalInput")
    vT = nc.dram_tensor("vT", [DIM, TS], DT_B, kind="ExternalInput")
    wq = nc.dram_tensor("wq", [DIM, H * DK], DT_B, kind="ExternalInput")
    wk = nc.dram_tensor("wk", [DIM, H * DK], DT_B, kind="ExternalInput")
    wv = nc.dram_tensor("wv", [DIM, H * DK], DT_B, kind="ExternalInput")
    pw = nc.dram_tensor("pwT", [H * DK, DIM], DT_B, kind="ExternalInput")
    pb = nc.dram_tensor("pb", [P, NDCH], DT_F, kind="ExternalInput")
    yT = nc.dram_tensor("yT", [DIM, TQ], DT_F, kind="ExternalOutput")

    def bcast_ap(ap, count):
        return bass.AP(tensor=ap.tensor, offset=ap.offset,
                       ap=[[0, count]] + [list(x) for x in ap.ap[1:]])

    with tile.TileContext(nc) as tc, \
         tc.tile_pool(name="l1", bufs=1) as l1, \
         tc.tile_pool(name="exp_pool", bufs=8) as expp, \
         tc.tile_pool(name="spsum", bufs=1, space="PSUM") as sps, \
         tc.tile_pool(name="oapsum", bufs=2, space="PSUM") as oaps, \
         tc.tile_pool(name="obpsum", bufs=2, space="PSUM") as obps, \
         tc.tile_pool(name="sums_pool", bufs=2) as smp, \
         tc.tile_pool(name="bc_pool", bufs=2) as bcp, \
         tc.tile_pool(name="tmpb_pool", bufs=2) as tbp, \
         tc.tile_pool(name="bounce", bufs=4, space="DRAM") as bncp:

        # ---- whole-program tiles ----
        kht = [l1.tile([P, TS], DT_B, name=f"kht{i}") for i in range(NHP)]
        qht = [l1.tile([P, TQ], DT_B, name=f"qht{i}") for i in range(NHP)]
        vhp = [l1.tile([P, H, DK + 1], DT_B, name=f"vhp{i}")
               for i in range(NST)]
        oall = [l1.tile([P, TQ], DT_B, name=f"oall{i}") for i in range(NHP)]
        pbt = l1.tile([P, NDCH], DT_F, name="pbt")
        nc.sync.dma_start(out=pbt[:, :], in_=pb[:, :])

        # ---------- emit helpers ----------
        def emit_kproj(hp, kin, wkt):
            for n in range(2):
                tag = "scA" if n == 0 else "scB"
                ps = sps.tile([P, TQ], DT_F, name=f"kps_{hp}_{n}", tag=tag)
                for d in range(NDCH):
                    for m in range(2):
                        nc.tensor.matmul(
                            ps[:, m * 512:(m + 1) * 512],
                            wkt[d][:, hp * P:(hp + 1) * P],
                            kin[d][:, n * TQ + m * 512:n * TQ + (m + 1) * 512],
                            start=(d == 0), stop=(d == NDCH - 1))
                nc.vector.tensor_copy(
                    kht[hp][:, n * TQ:(n + 1) * TQ], ps[:, :])

        def emit_qproj(hp, qin, wqt):
            ps = sps.tile([P, TQ], DT_F, name=f"qps_{hp}", tag="scA")
            for d in range(NDCH):
                for m in range(2):
                    nc.tensor.matmul(
                        ps[:, m * 512:(m + 1) * 512],
                        wqt[d][:, hp * P:(hp + 1) * P],
                        qin[d][:, m * 512:(m + 1) * 512],
                        start=(d == 0), stop=(d == NDCH - 1))
            nc.vector.tensor_copy(qht[hp][:, :], ps[:, :])

        def emit_vproj(st, half, vin, wvt):
            # vhp[st][s, half*8:(half+1)*8, 0:64] for 8 heads
            tag = "scA" if st % 2 == 0 else "scB"
            ps = sps.tile([P, TQ], DT_F, name=f"vps_{st}_{half}", tag=tag)
            for d in range(NDCH):
                nc.tensor.matmul(
                    ps[:, 0:512],
                    vin[d][:, st * P:(st + 1) * P],
                    wvt[d][:, half * 512:(half + 1) * 512],
                    start=(d == 0), stop=(d == NDCH - 1))
            nc.vector.tensor_copy(
                vhp[st][:, half * 8:(half + 1) * 8, 0:DK],
                ps[:, 0:512].rearrange("p (h d) -> p h d", d=DK))
            nc.vector.memset(
                vhp[st][:, half * 8:(half + 1) * 8, DK:DK + 1], 1.0)

        def emit_block(hp):
            """scores + exp + PV for head pair hp, kt-interleaved."""
            hA, hB = 2 * hp, 2 * hp + 1
            opA = [oaps.tile([DK + 1, 512], DT_F, name=f"opA_{hp}_{q}",
                             tag="opA") for q in range(2)]
            opB = [obps.tile([DK + 1, 512], DT_F, name=f"opB_{hp}_{q}",
                             tag="opB") for q in range(2)]
            for kt in range(NST):
                sa = sps.tile([P, TQ], DT_F, name=f"scA_{hp}_{kt}",
                              tag="scA")
                sb = sps.tile([P, TQ], DT_F, name=f"scB_{hp}_{kt}",
                              tag="scB")
                for m in range(2):
                    nc.tensor.matmul(sa[:, m * 512:(m + 1) * 512],
                                     kht[hp][0:DK, kt * P:(kt + 1) * P],
                                     qht[hp][0:DK, m * 512:(m + 1) * 512],
                                     start=True, stop=True)
                    nc.tensor.matmul(sb[:, m * 512:(m + 1) * 512],
                                     kht[hp][DK:P, kt * P:(kt + 1) * P],
                                     qht[hp][DK:P, m * 512:(m + 1) * 512],
                                     start=True, stop=True)
                ea = expp.tile([P, TQ], DT_B, name=f"expA_{hp}_{kt}",
                               tag="exp")
                eb = expp.tile([P, TQ], DT_B, name=f"expB_{hp}_{kt}",
                               tag="exp")
                nc.scalar.activation(ea[:, :], sa[:, :], AF.Exp,
                                     scale=1.0 / 32.0)
                nc.scalar.activation(eb[:, :], sb[:, :], AF.Exp,
                                     scale=1.0 / 32.0)
                for q in range(2):
                    nc.tensor.matmul(
                        opA[q][:, :], vhp[kt][:, hA, :],
                        ea[:, q * 512:(q + 1) * 512],
                        start=(kt == 0), stop=(kt == NST - 1))
                for q in range(2):
                    nc.tensor.matmul(
                        opB[q][:, :], vhp[kt][:, hB, :],
                        eb[:, q * 512:(q + 1) * 512],
                        start=(kt == 0), stop=(kt == NST - 1))
            for side, ops_ in ((0, opA), (1, opB)):
                h = 2 * hp + side
                for q in range(2):
                    op = ops_[q]
                    sm = smp.tile([DK + 1, 512], DT_F, name=f"sm_{h}_{q}",
                                  tag="sm")
                    nc.vector.reciprocal(sm[DK:DK + 1, :],
                                         op[DK:DK + 1, :])
                    bn = bncp.tile([1, 512], DT_F, name=f"bn_{h}_{q}",
                                   tag="bn")
                    nc.sync.dma_start(out=bn[:, :], in_=sm[DK:DK + 1, :])
                    bc = bcp.tile([DK, 512], DT_F, name=f"bc_{h}_{q}",
                                  tag="bc")
                    nc.sync.dma_start(out=bc[:, :],
                                      in_=bcast_ap(bn[0:1, :], DK))
                    if side == 0:
                        nc.vector.tensor_mul(
                            oall[hp][0:DK, q * 512:(q + 1) * 512],
                            op[0:DK, :], bc[:, :])
                    else:
                        tb_ = tbp.tile([DK, 512], DT_B, name=f"tb_{h}_{q}",
                                       tag="tb")
                        nc.vector.tensor_mul(tb_[:, :], op[0:DK, :],
                                             bc[:, :])
                        nc.gpsimd.dma_start(
                            out=oall[hp][DK:P, q * 512:(q + 1) * 512],
                            in_=tb_[:, :])

        # ---------- program ----------
        # V-proj lo half (heads 0..7) first so PV(0..3) can run early.
        with tc.tile_pool(name="v_in1", bufs=1) as vip:
            vin = [vip.tile([P, TS], DT_B, name=f"vin{d}")
                   for d in range(NDCH)]
            wvt = [vip.tile([P, H * DK], DT_B, name=f"wvt{d}")
                   for d in range(NDCH)]
            for d in range(NDCH):
                nc.sync.dma_start(out=vin[d][:, :],
                                  in_=vT[d * P:(d + 1) * P, :])
                nc.scalar.dma_start(out=wvt[d][:, :],
                                    in_=wv[d * P:(d + 1) * P, :])
            for st in range(NST):
                emit_vproj(st, 0, vin, wvt)

        with tc.tile_pool(name="kq_in", bufs=1) as kqp:
            kin = [kqp.tile([P, TS], DT_B, name=f"kin{d}")
                   for d in range(NDCH)]
            wkt = [kqp.tile([P, H * DK], DT_B, name=f"wkt{d}")
                   for d in range(NDCH)]
            qin = [kqp.tile([P, TQ], DT_B, name=f"qin{d}")
                   for d in range(NDCH)]
            wqt = [kqp.tile([P, H * DK], DT_B, name=f"wqt{d}")
                   for d in range(NDCH)]
            for d in range(NDCH):
                nc.sync.dma_start(out=kin[d][:, :],
                                  in_=kT[d * P:(d + 1) * P, :])
                nc.scalar.dma_start(out=wkt[d][:, :],
                                    in_=wk[d * P:(d + 1) * P, :])
                nc.gpsimd.dma_start(out=qin[d][:, :],
                                    in_=qT[d * P:(d + 1) * P, :])
                nc.gpsimd.dma_start(out=wqt[d][:, :],
                                    in_=wq[d * P:(d + 1) * P, :])

            for hp in range(4):
                emit_kproj(hp, kin, wkt)
                emit_qproj(hp, qin, wqt)
                emit_block(hp)
                emit_kproj(hp + 4, kin, wkt)
                emit_qproj(hp + 4, qin, wqt)

        # V-proj hi half (heads 8..15), then blocks 4..7.
        with tc.tile_pool(name="v_in2", bufs=1) as vip2:
            vin2 = [vip2.tile([P, TS], DT_B, name=f"vin2_{d}")
                    for d in range(NDCH)]
            wvt2 = [vip2.tile([P, H * DK], DT_B, name=f"wvt2_{d}")
                    for d in range(NDCH)]
            for d in range(NDCH):
                nc.sync.dma_start(out=vin2[d][:, :],
                                  in_=vT[d * P:(d + 1) * P, :])
                nc.sync.dma_start(out=wvt2[d][:, :],
                                  in_=wv[d * P:(d + 1) * P, :])
            for st in range(NST):
                emit_vproj(st, 1, vin2, wvt2)
            emit_block(4)
            emit_block(5)

        with tc.tile_pool(name="pw_pool", bufs=1) as pwp:
            pwsb = [pwp.tile([P, DIM], DT_B, name=f"pwsb{i}")
                    for i in range(NHP)]
            for hp in range(NHP):
                nc.gpsimd.dma_start(out=pwsb[hp][:, :],
                                    in_=pw[hp * P:(hp + 1) * P, :])
            emit_block(6)
            emit_block(7)

            # ---- phase C ----
            with tc.tile_pool(name="yst_pool", bufs=2) as ystp:
                for dt_ in range(NDCH):
                    tag = "scA" if dt_ % 2 == 0 else "scB"
                    ps = sps.tile([P, TQ], DT_F, name=f"yps_{dt_}",
                                  tag=tag)
                    for hp in range(NHP):
                        for m in range(2):
                            nc.tensor.matmul(
                                ps[:, m * 512:(m + 1) * 512],
                                pwsb[hp][:, dt_ * P:(dt_ + 1) * P],
                                oall[hp][:, m * 512:(m + 1) * 512],
                                start=(hp == 0), stop=(hp == NHP - 1))
                    yst = ystp.tile([P, TQ], DT_F, name=f"yst_{dt_}",
                                    tag="yst")
                    nc.vector.tensor_scalar_add(yst[:, :], ps[:, :],
                                                pbt[:, dt_:dt_ + 1])
                    nc.sync.dma_start(
                        out=yT[dt_ * P:(dt_ + 1) * P, :], in_=yst[:, :])

    nc.compile()
    return nc


def kernel(q, k, v, w_q, w_k, w_v, proj_w, proj_b):
    global _NC, LAST_RESULT
    import ml_dtypes
    from concourse.bass_utils import run_bass_kernel_spmd

    if _NC is None:
        _NC = _build()

    bf16 = ml_dtypes.bfloat16
    q = np.asarray(q, dtype=np.float32)
    k = np.asarray(k, dtype=np.float32)
    v = np.asarray(v, dtype=np.float32)
    w_q = np.asarray(w_q, dtype=np.float32)
    w_k = np.asarray(w_k, dtype=np.float32)
    w_v = np.asarray(w_v, dtype=np.float32)
    proj_w = np.asarray(proj_w, dtype=np.float32)
    proj_b = np.asarray(proj_b, dtype=np.float32)

    wq2 = np.ascontiguousarray(
        np.transpose(w_q, (1, 0, 2)).reshape(DIM, H * DK)).astype(bf16)
    wk2 = np.ascontiguousarray(
        np.transpose(w_k, (1, 0, 2)).reshape(DIM, H * DK)).astype(bf16)
    wv2 = np.ascontiguousarray(
        np.transpose(w_v, (1, 0, 2)).reshape(DIM, H * DK)).astype(bf16)
    pwT = np.ascontiguousarray(proj_w.T).astype(bf16)
    pb2 = np.ascontiguousarray(proj_b.reshape(NDCH, P).T)

    in_maps = []
    for c in range(N_CORES):
        b, qo = c // 2, c % 2
        if qo == 0:
            kTb = np.ascontiguousarray(k[b].T).astype(bf16)
            vTb = np.ascontiguousarray(v[b].T).astype(bf16)
        in_maps.append({
            "qT": np.ascontiguousarray(
                q[b, qo * TQ:(qo + 1) * TQ, :].T).astype(bf16),
            "kT": kTb,
            "vT": vTb,
            "wq": wq2, "wk": wk2, "wv": wv2,
            "pwT": pwT, "pb": pb2,
        })

    res = run_bass_kernel_spmd(_NC, in_maps, list(range(N_CORES)), trace=TRACE)
    LAST_RESULT = res

    out = np.empty((B, L, DIM), dtype=np.float32)
    for c in range(N_CORES):
        b, qo = c // 2, c % 2
        out[b, qo * TQ:(qo + 1) * TQ, :] = res.results[c]["yT"].T
    return out
